# revision 1
# baseline (speedup 1.0000x reference)
"""DCNv2 deformable-conv alignment kernel for 8 Trainium2 NeuronCores (v2).

Sharding: core i handles (b = i//2, row-half = i%2) of the B=4, H=128 input.

v2 redesign vs baseline:
- all matmuls in bf16 (4x PE throughput vs fp32)
- guard-band padded gather source (no validity masks; OOB corners read zeros)
- pair-gather: source is row-interleaved [A|B] pair copies in bf16, gather
  d=2 fetches an x-adjacent pair per index -> half the indices, half bytes
- per-chunk banded gather source view (32 rows) to cut num_elems
- index wrap for the gather is ONE contiguous SBUF->SBUF DMA per tap
  (c-major index layout chosen so the wrap is 128B-contiguous runs)
- corner weights replicated 8->128 partitions by a broadcast DMA (not matmul)
- corner reduction folded into the DCN matmul (4 accumulating matmuls/tap)
- positive-shifted coordinates so mod(x,1) == frac(x) on DVE
"""
import sys

for _p in ("/opt/trn_rl_repo", "/root/.axon_site/_ro/trn_rl_repo"):
    if _p not in sys.path:
        sys.path.insert(0, _p)

import numpy as np
import ml_dtypes

NF, G, K = 64, 8, 3
KK = K * K
CG = NF // G
B, H, W = 4, 128, 128
N_CORES = 8
HALF = H // 2

GP = 14                 # guard pad (rows/cols) around each core's band
WP = W + 2 * GP         # 156 padded width
HP = HALF + 2 * GP      # 92 padded rows
NPR = WP                # pairs per padded row (78 A-pairs + 78 B-pairs)
NE_AB = HP * NPR        # 14352 total pair elements
ROWS_BAND = 32
BAND = ROWS_BAND * NPR  # 4992 pair elems per gather view

CH = 512                # positions per chunk (4 rows)
RPC = CH // W           # 4
NCHUNK = HALF * W // CH # 16
NIDX = CH               # 512 gather indices per tap (d=4 fetches 2x2 patch)
ROWS_B4 = 27
BAND4 = ROWS_B4 * NPR   # patch slots per chunk band

IDX_ON_ACT = True       # emit idx int16 conversion on scalar engine
FLOOR_TRUNC = False     # assume fp32->int32 cast truncates (test on HW)
CAST_ON_ACT = False     # run fp32<->int32 casts on the scalar engine

_compiled = None


def _build_program():
    import concourse.bacc as bacc
    import concourse.mybir as mybir
    import concourse.tile as tile
    from concourse.tile_rust import add_dep_helper
    from concourse.bass_types import AP

    def mk_ap(base_ap, dims):
        """Raw AP on base_ap's tensor+offset with explicit [stride, count]
        dims in flat element units (partition stride = row width)."""
        return AP(base_ap.tensor, base_ap.offset, dims)

    dt = mybir.dt
    Alu = mybir.AluOpType
    Act = mybir.ActivationFunctionType
    f32 = dt.float32
    f32r = dt.float32r
    bf16 = dt.bfloat16

    nc = bacc.Bacc("TRN2", target_bir_lowering=False, debug=False,
                   num_devices=N_CORES)

    # ---- DRAM I/O ----
    conv_in_d = nc.dram_tensor("conv_in", [128, 68 * 130], bf16, kind="ExternalInput").ap()
    p4_d = nc.dram_tensor("p4", [128, NE_AB * 4], bf16, kind="ExternalInput").ap()
    w1_d = nc.dram_tensor("w1", [128, KK * 64], bf16, kind="ExternalInput").ap()
    w2_d = nc.dram_tensor("w2", [64, 3 * KK * 72], bf16, kind="ExternalInput").ap()
    w3_d = nc.dram_tensor("w3", [128, KK * 64], bf16, kind="ExternalInput").ap()
    wrep_d = nc.dram_tensor("wrep", [72, KK * 128], f32r, kind="ExternalInput").ap()
    by_d = nc.dram_tensor("by", [72, 1], f32, kind="ExternalInput").ap()
    bx_d = nc.dram_tensor("bx", [72, 1], f32, kind="ExternalInput").ap()
    bm_d = nc.dram_tensor("bm", [72, 1], f32, kind="ExternalInput").ap()
    b1_d = nc.dram_tensor("b1", [64, 1], f32, kind="ExternalInput").ap()
    b3_d = nc.dram_tensor("b3", [64, 1], f32, kind="ExternalInput").ap()
    e0_d = nc.dram_tensor("e0", [64, 1], f32, kind="ExternalInput").ap()
    e65_d = nc.dram_tensor("e65", [64, 1], f32, kind="ExternalInput").ap()
    rampc_d = nc.dram_tensor("rampc", [128, CH], f32, kind="ExternalInput").ap()
    c156_d = nc.dram_tensor("c156", [72, 1], f32, kind="ExternalInput").ap()
    out_d = nc.dram_tensor("out", [64, HALF * W], f32, kind="ExternalOutput").ap()

    def lrelu_stt(out_ap, in_ap):
        nc.vector.scalar_tensor_tensor(out_ap, in_ap, 0.1, in_ap, Alu.mult, Alu.max)

    # Static SBUF for gather-adjacent tensors (partition-strided DMA writes
    # are under-tracked by Tile): fixed addresses + explicit deps.
    # Taps are processed in groups of TG=3: one gather per group; corner
    # weights are replicated 8->128 partitions by a one-hot fp32r matmul.
    NIC = KK * NIDX          # 4608 indices per per-chunk gather
    idxw_s = [nc.alloc_sbuf_tensor(f"idxw{i}", [128, NIC // 16], dt.int16).ap()
              for i in range(2)]
    gout_one = nc.alloc_sbuf_tensor("gout0", [128, NIC * 4], bf16).ap()

    with tile.TileContext(nc) as tc:
        with tc.tile_pool(name="const", bufs=1) as cpool, \
             tc.tile_pool(name="psum", bufs=2, space="PSUM") as ppool, \
             tc.tile_pool(name="psum_rep", bufs=1, space="PSUM") as ppool_r:

            # ---- persistent loads ----
            w1_sb = cpool.tile([128, KK * 64], bf16)
            nc.sync.dma_start(w1_sb[:], w1_d[:])
            w2_sb = cpool.tile([64, 3 * KK * 72], bf16)
            nc.sync.dma_start(w2_sb[:], w2_d[:])
            w3_sb = cpool.tile([128, KK * 64], bf16)
            nc.sync.dma_start(w3_sb[:], w3_d[:])
            wrep_sb = cpool.tile([72, KK * 128], f32r)
            nc.sync.dma_start(wrep_sb[:], wrep_d[:])
            by_sb = cpool.tile([72, 1], f32)
            nc.sync.dma_start(by_sb[:], by_d[:])
            bx_sb = cpool.tile([72, 1], f32)
            nc.sync.dma_start(bx_sb[:], bx_d[:])
            bm_sb = cpool.tile([72, 1], f32)
            nc.sync.dma_start(bm_sb[:], bm_d[:])
            b1_sb = cpool.tile([64, 1], f32)
            nc.sync.dma_start(b1_sb[:], b1_d[:])
            b3_sb = cpool.tile([64, 1], f32)
            nc.sync.dma_start(b3_sb[:], b3_d[:])
            e0_sb = cpool.tile([64, 1], f32)
            nc.sync.dma_start(e0_sb[:], e0_d[:])
            e65_sb = cpool.tile([64, 1], f32)
            nc.sync.dma_start(e65_sb[:], e65_d[:])
            rampc_sb = cpool.tile([128, CH], f32)
            nc.sync.dma_start(rampc_sb[:], rampc_d[:])
            c156_sb = cpool.tile([72, 1], f32)
            nc.sync.dma_start(c156_sb[:], c156_d[:])

            # ---- conv1 -> off_feat rows [-1, HALF+1), 130-wide padded, bf16 ----
            off_sb = cpool.tile([64, 66 * 130], bf16)
            nc.vector.memset(off_sb[:], 0.0)
            off_v = off_sb[:].rearrange("p (r c) -> p r c", c=130)
            with tc.tile_pool(name="cin", bufs=1) as cinpool:
                conv_in_sb = cinpool.tile([128, 68 * 130], bf16)
                nc.sync.dma_start(conv_in_sb[:], conv_in_d[:])
                cin_v = conv_in_sb[:].rearrange("p (r c) -> p r c", c=130)
                j0 = 0
                while j0 < 66:
                    nrow = min(4, 66 - j0)
                    ps1 = ppool.tile([64, nrow, 128], f32, tag="dcn_ps")
                    for kt in range(KK):
                        ky, kx = kt // 3, kt % 3
                        rhs = cin_v[:, j0 + ky: j0 + ky + nrow, kx: kx + 128]
                        nc.tensor.matmul(ps1[:], w1_sb[:, kt * 64:(kt + 1) * 64],
                                         rhs, start=(kt == 0), stop=(kt == KK - 1))
                    scf = cinpool.tile([64, nrow, 128], f32, tag="scf")
                    nc.vector.tensor_scalar(scf[:], ps1[:], b1_sb[:, 0:1], None, Alu.add)
                    lrelu_stt(off_v[:, j0: j0 + nrow, 1:129], scf[:])
                    j0 += nrow
            # zero off rows outside the image (conv2 zero-padding semantics)
            nc.vector.tensor_scalar(off_sb[:, 0:130], off_sb[:, 0:130],
                                    e0_sb[:, 0:1], None, Alu.mult)
            nc.vector.tensor_scalar(off_sb[:, 65 * 130:66 * 130],
                                    off_sb[:, 65 * 130:66 * 130],
                                    e65_sb[:, 0:1], None, Alu.mult)

            # ---- per-chunk pipeline ----
            gathers, mults, dcn_last, wraps, reps = [], [], [], [], []
            with tc.tile_pool(name="work", bufs=1) as wpool, \
                 tc.tile_pool(name="work2", bufs=2) as wpool2, \
                 tc.tile_pool(name="workp4", bufs=1) as wpool_p4:
                for c in range(NCHUNK):
                    # conv2: one shared psum tag cycled across the 3 fields
                    outs_f = []
                    for f, (bias_ap, fun, tagn) in enumerate(
                            ((by_sb, Act.Identity, "qy"),
                             (bx_sb, Act.Identity, "qx"),
                             (bm_sb, Act.Sigmoid, "msk"))):
                        psf = ppool.tile([72, RPC, 128], f32, tag="ps2")
                        for kt in range(KK):
                            ky, kx = kt // 3, kt % 3
                            rhs = off_v[:, c * RPC + ky: c * RPC + ky + RPC, kx: kx + 128]
                            nc.tensor.matmul(
                                psf[:],
                                w2_sb[:, (f * KK + kt) * 72:(f * KK + kt + 1) * 72],
                                rhs, start=(kt == 0), stop=(kt == KK - 1))
                        ot = wpool.tile([72, CH], f32, tag=tagn)
                        nc.scalar.activation(ot[:], psf[:].rearrange("p a b -> p (a b)"),
                                             fun, bias=bias_ap[:, 0:1], scale=1.0)
                        outs_f.append(ot)
                    qy, qx, msk = outs_f

                    # frac/floor via int cast (coords strictly positive).
                    # FLOOR_TRUNC assumes the fp32->int32 cast truncates; the
                    # exact path corrects for round-to-nearest converters.
                    def cast_int_back(q, tag):
                        """round/trunc q -> int32 -> f32, on ACT or DVE."""
                        ti = wpool.tile([72, CH], dt.int32, tag="fl_i32")
                        tf = wpool.tile([72, CH], f32, tag=tag + "_f")
                        if CAST_ON_ACT:
                            nc.scalar.activation(ti[:], q[:], Act.Identity,
                                                 bias=0.0, scale=1.0)
                            nc.scalar.activation(tf[:], ti[:], Act.Identity,
                                                 bias=0.0, scale=1.0)
                        else:
                            nc.vector.tensor_copy(ti[:], q[:])
                            nc.vector.tensor_copy(tf[:], ti[:])
                        return tf

                    def floor_frac(q, tag):
                        tf = cast_int_back(q, tag)
                        if FLOOR_TRUNC:
                            fl = tf
                        else:
                            gg = wpool.tile([72, CH], f32, tag="fl_gt")
                            nc.vector.tensor_tensor(gg[:], tf[:], q[:], Alu.is_gt)
                            fl = wpool.tile([72, CH], f32, tag=tag)
                            nc.vector.tensor_tensor(fl[:], tf[:], gg[:], Alu.subtract)
                        w = wpool.tile([72, CH], f32, tag=tag + "_w")
                        nc.vector.tensor_tensor(w[:], q[:], fl[:], Alu.subtract)
                        return fl, w

                    fyf, wy = floor_frac(qy, "fy")
                    fxf, wx = floor_frac(qx, "fx")

                    # pair index: pidx = fyf*156 + (ramp + fxf*0.5) + 155*parh
                    # parh = frac(hx) in {0, 0.5}; |hx - round(hx)| is correct
                    # for either converter rounding mode.
                    hx = wpool.tile([72, CH], f32, tag="hx")
                    nc.vector.scalar_tensor_tensor(hx[:], fxf[:], 0.5,
                                                   rampc_sb[:72, :], Alu.mult, Alu.add)
                    pf_ = cast_int_back(hx, "pr")
                    pd = wpool.tile([72, CH], f32, tag="pr_d")
                    nc.vector.tensor_tensor(pd[:], hx[:], pf_[:], Alu.subtract)
                    # pd in {0, +-0.5}; pd^2 in {0, 0.25} -> parity term
                    sq = wpool.tile([72, CH], f32, tag="sq")
                    nc.vector.tensor_tensor(sq[:], pd[:], pd[:], Alu.mult)
                    hxp = wpool.tile([72, CH], f32, tag="hxp")
                    nc.vector.scalar_tensor_tensor(hxp[:], sq[:], float(2 * (NPR - 1)),
                                                   hx[:], Alu.mult, Alu.add)
                    pidx = wpool.tile([72, CH], f32, tag="pidx")
                    nc.vector.scalar_tensor_tensor(pidx[:], fyf[:], float(NPR),
                                                   hxp[:], Alu.mult, Alu.add)

                    # idx2: patch-slot indices int16 [72, CH]; band start (4c
                    # rows) cancels the chunk shift.
                    idx2 = wpool2.tile([72, CH], dt.int16, tag="idx2")
                    idx_ins = []
                    if IDX_ON_ACT:
                        d = nc.scalar.activation(idx2[:], pidx[:],
                                                 Act.Identity, bias=0.0, scale=1.0)
                    else:
                        d = nc.vector.tensor_scalar(idx2[:], pidx[:],
                                                    0.0, None, Alu.add)
                    # idx2 slot reuse vs the (untracked) wrap-DMA reads of
                    # the chunk two back (KK wrap DMAs per chunk)
                    for w in wraps[(c - 2) * KK:(c - 1) * KK] if c >= 2 else []:
                        add_dep_helper(d.ins, w.ins, True, "idx2 WAR wrap")
                    idx_ins.append(d)

                    # corner weights -> cu_store bf16, layout col =
                    # xl*32 + yc*16 + r*4 + xh*2 + xc  (matches gather j-order)
                    t = wpool.tile([72, CH], f32, tag="t")
                    nc.vector.tensor_tensor(t[:], wy[:], msk[:], Alu.mult)
                    uy0 = wpool.tile([72, CH], f32, tag="uy0")
                    nc.vector.tensor_tensor(uy0[:], msk[:], t[:], Alu.subtract)
                    ux0 = wpool.tile([72, CH], f32, tag="ux0")
                    nc.scalar.activation(ux0[:], wx[:], Act.Identity, bias=1.0, scale=-1.0)

                    cu_store = wpool2.tile([72, CH * 4], f32r, tag="cu")
                    # j-ordered flat layout: col = xl*64 + r*16 + xh*4 + yc*2 + xc
                    cu_v = cu_store[:].rearrange(
                        "p (xl r xh yc xc) -> p xl r xh yc xc",
                        xl=32, r=RPC, xh=4, yc=2, xc=2).transpose(
                        [0, 4, 5, 2, 3, 1])
                    cu_ins = []
                    for (yc, xc, a_ap, b_ap) in (
                            (0, 0, uy0, ux0), (0, 1, uy0, wx),
                            (1, 0, t, ux0), (1, 1, t, wx)):
                        d = nc.vector.tensor_tensor(
                            cu_v[:, yc, xc],
                            a_ap[:].rearrange("p (r xh xl) -> p r xh xl",
                                              r=RPC, xh=4),
                            b_ap[:].rearrange("p (r xh xl) -> p r xh xl",
                                              r=RPC, xh=4),
                            Alu.mult)
                        cu_ins.append(d)

                    # DMA the 2x2-patch band for this chunk from DRAM,
                    # split across three DGE queues for bandwidth
                    p4 = wpool_p4.tile([128, BAND4 * 4], bf16, tag="p4")
                    base_e = c * RPC * NPR * 4
                    # sync/scalar take a third each; the gpsimd third is
                    # further split into 4 sub-DMAs - SWDGE dispatches keep
                    # the Q7 warm between ap_gather invocations
                    third = (BAND4 * 4) // 3
                    cuts = [0, third, 2 * third]
                    gp_lo, gp_hi = 2 * third, BAND4 * 4
                    qs = [(nc.sync, 0, third), (nc.scalar, third, 2 * third)]
                    step = (gp_hi - gp_lo) // 4
                    for si in range(4):
                        lo = gp_lo + si * step
                        hi = gp_lo + (si + 1) * step if si < 3 else gp_hi
                        qs.append((nc.gpsimd, lo, hi))
                    p4_dmas = []
                    for q, lo, hi in qs:
                        dd = q.dma_start(p4[:, lo:hi],
                                         p4_d[:, base_e + lo: base_e + hi])
                        if gathers:
                            add_dep_helper(dd.ins, gathers[-1].ins, True,
                                           "p4 WAR prev gather")
                        p4_dmas.append(dd)
                    src_ap = p4[:]

                    dcn_ps = ppool.tile([64, CH], f32, tag="dcn_ps")
                    bi = c % 2
                    grp_wraps = []
                    for kt in range(KK):
                        # wrap DMA: idxw[16m+prow, kt*32+col] <-
                        #   idx2[kt*8+m, prow*32+col]   (3-dim manual APs)
                        d_b = idxw_s[bi][:, kt * 32:(kt + 1) * 32]
                        s_b = idx2[kt * 8:(kt + 1) * 8, :]
                        q = nc.sync if (len(wraps) % 2 == 0) else nc.scalar
                        wrap = q.dma_start(
                            mk_ap(d_b, [[NIC // 16, 128], [1, 32]]),
                            mk_ap(s_b, [[NIDX, 8], [32, 16], [1, 32]]))
                        for d in idx_ins:
                            add_dep_helper(wrap.ins, d.ins, True, "wrap RAW idx2")
                        if len(gathers) >= 2:
                            add_dep_helper(wrap.ins, gathers[-2].ins, True,
                                           "idxw WAR prev gather")
                        wraps.append(wrap)
                        grp_wraps.append(wrap)

                    gth = nc.gpsimd.ap_gather(
                        out_ap=gout_one[:], in_ap=src_ap,
                        idxs_ap=idxw_s[bi][:], channels=128,
                        num_elems=BAND4, d=4, num_idxs=NIC)
                    for w in grp_wraps:
                        add_dep_helper(gth.ins, w.ins, True, "gather RAW idxw")
                    for dd in p4_dmas:
                        add_dep_helper(gth.ins, dd.ins, True, "gather RAW p4")
                    if mults:
                        # gout is single-buffered: wait for the previous
                        # chunk's last reader (DVE in-order covers all)
                        add_dep_helper(gth.ins, mults[-1].ins, True,
                                       "gout WAR prev mults")
                    gathers.append(gth)

                    for kt in range(KK):
                        # replicate cu rows (k,g) -> 16 partitions each via
                        # one-hot fp32r matmul (1 cyc/row on PE)
                        rep_ps = ppool_r.tile([128, CH * 4], f32, tag="rep_ps")
                        for h in range(4):
                            nc.tensor.matmul(
                                rep_ps[:, h * CH:(h + 1) * CH],
                                wrep_sb[:, kt * 128:(kt + 1) * 128],
                                cu_store[:, h * CH:(h + 1) * CH],
                                start=True, stop=True)
                        # weighting: gw = gout(tap slice) * rep  (bf16 out)
                        gw = wpool2.tile([128, CH * 4], bf16, tag="gw")
                        mult = nc.vector.tensor_tensor(
                            gw[:], gout_one[:, kt * CH * 4:(kt + 1) * CH * 4],
                            rep_ps[:], Alu.mult)
                        add_dep_helper(mult.ins, gth.ins, True, "mult RAW gout")
                        mults.append(mult)

                        gw_v = gw[:].rearrange(
                            "p (xl r xh yc xc) -> p xl r xh yc xc",
                            xl=32, r=RPC, xh=4, yc=2, xc=2).transpose(
                            [0, 4, 5, 2, 3, 1])
                        for f in range(4):
                            nc.tensor.matmul(
                                dcn_ps[:], w3_sb[:, kt * 64:(kt + 1) * 64],
                                gw_v[:, f // 2, f % 2],
                                start=(kt == 0 and f == 0),
                                stop=(kt == KK - 1 and f == 3))

                    oc = wpool.tile([64, CH], f32, tag="oc")
                    nc.scalar.activation(oc[:], dcn_ps[:], Act.Identity,
                                         bias=b3_sb[:, 0:1], scale=1.0)
                    ob = wpool.tile([64, CH], f32, tag="ob")
                    lrelu_stt(ob[:], oc[:])
                    nc.sync.dma_start(out_d[:, c * CH:(c + 1) * CH], ob[:])

    nc.compile()
    return nc


def _prep_inputs(nbr, ref, w_off1, b_off1, w_om, b_om, w_dcn, b_dcn):
    """Build the 8 per-core input dicts."""
    bf = ml_dtypes.bfloat16
    in_maps = []
    # weights shared by all cores
    w1 = np.zeros((128, KK * 64), np.float32)
    for kt in range(KK):
        ky, kx = kt // 3, kt % 3
        w1[:, kt * 64:(kt + 1) * 64] = w_off1[:, :, ky, kx].T
    w2 = np.zeros((64, 3 * KK * 72), np.float32)
    for f in range(3):
        for kt in range(KK):
            ky, kx = kt // 3, kt % 3
            blk = np.zeros((64, 72), np.float32)
            for k in range(KK):
                for g in range(G):
                    blk[:, k * 8 + g] = w_om[f * 72 + g * KK + k, :, ky, kx]
            w2[:, (f * KK + kt) * 72:(f * KK + kt + 1) * 72] = blk
    w3 = np.zeros((128, KK * 64), np.float32)
    wd = w_dcn.reshape(64, G, CG, 3, 3)
    for kt in range(KK):
        ky, kx = kt // 3, kt % 3
        blk = np.zeros((128, 64), np.float32)
        for g in range(G):
            for j in range(CG):
                blk[16 * g + j, :] = wd[:, g, j, ky, kx]
        w3[:, kt * 64:(kt + 1) * 64] = blk

    wrep = np.zeros((72, KK * 128), np.float32)
    for kt in range(KK):
        for m in range(128):
            wrep[kt * 8 + m // 16, kt * 128 + m] = 1.0

    dy = np.repeat(np.arange(3) - 1, 3).astype(np.float32)
    dx = np.tile(np.arange(3) - 1, 3).astype(np.float32)
    by = np.zeros((72, 1), np.float32)
    bx = np.zeros((72, 1), np.float32)
    bm = np.zeros((72, 1), np.float32)
    for k in range(KK):
        for g in range(G):
            p = k * 8 + g
            by[p, 0] = b_om[0 * 72 + g * KK + k] + dy[k] + GP
            bx[p, 0] = b_om[1 * 72 + g * KK + k] + dx[k] + GP
            bm[p, 0] = b_om[2 * 72 + g * KK + k]
    b1 = b_off1.reshape(64, 1).astype(np.float32)
    b3 = b_dcn.reshape(64, 1).astype(np.float32)

    # ramp for chunk 0: y_local*NPR + x/2 over positions q = r*128 + x
    q = np.arange(CH, dtype=np.float32)
    rampc = (q // W) * NPR + (q % W) * 0.5
    rampc = np.broadcast_to(rampc, (128, CH)).astype(np.float32).copy()

    for core in range(N_CORES):
        b = core // 2
        s = (core % 2) * HALF
        # conv1 input: concat channels, rows [s-2, s+66), zero pad, 130 cols
        ci = np.zeros((128, 68, 130), np.float32)
        cat = np.concatenate([nbr[b], ref[b]], axis=0)
        r_lo, r_hi = s - 2, s + 66
        src_lo, src_hi = max(r_lo, 0), min(r_hi, H)
        ci[:, src_lo - r_lo: src_hi - r_lo, 1:129] = cat[:, src_lo:src_hi, :]

        # gather source: guard-padded per-channel image, as 2x2 patches:
        # slot (y, s) = (pair[y, s], pair[y+1, s]) with A|B x-parity pairs
        pad = np.zeros((128, HP, WP), np.float32)
        g_lo, g_hi = s - GP, s + HALF + GP
        sg_lo, sg_hi = max(g_lo, 0), min(g_hi, H)
        for g in range(G):
            for j in range(16):
                ch = CG * g + (j % CG)
                pad[16 * g + j, sg_lo - g_lo: sg_hi - g_lo, GP:GP + W] = \
                    nbr[b, ch, sg_lo:sg_hi, :]
        shift = np.concatenate([pad[:, :, 1:], np.zeros((128, HP, 1), np.float32)],
                               axis=2)
        ab = np.concatenate([pad.reshape(128, HP, NPR // 2, 2),
                             shift.reshape(128, HP, NPR // 2, 2)], axis=2)
        nxt = np.concatenate([ab[:, 1:], np.zeros((128, 1, NPR, 2), np.float32)],
                             axis=1)
        patch4 = np.concatenate([ab, nxt], axis=3).reshape(128, NE_AB * 4)

        e0 = np.full((64, 1), 0.0 if s == 0 else 1.0, np.float32)
        e65 = np.full((64, 1), 0.0 if s + HALF == H else 1.0, np.float32)
        in_maps.append(dict(
            conv_in=ci.reshape(128, -1).astype(bf),
            p4=patch4.astype(bf),
            w1=w1.astype(bf), w2=w2.astype(bf), w3=w3.astype(bf), wrep=wrep,
            by=by, bx=bx, bm=bm, b1=b1, b3=b3, e0=e0, e65=e65,
            rampc=rampc, c156=np.full((72, 1), float(NPR), np.float32),
        ))
    return in_maps


def kernel(**inputs):
    global _compiled
    from concourse.bass_utils import run_bass_kernel_spmd

    if _compiled is None:
        _compiled = _build_program()
    nc = _compiled

    in_maps = _prep_inputs(
        inputs["nbr_fea_l"], inputs["ref_fea_l"], inputs["w_off1"],
        inputs["b_off1"], inputs["w_om"], inputs["b_om"],
        inputs["w_dcn"], inputs["b_dcn"])

    res = run_bass_kernel_spmd(nc, in_maps, core_ids=list(range(N_CORES)))
    out = np.zeros((B, NF, H, W), np.float32)
    for core in range(N_CORES):
        b = core // 2
        s = (core % 2) * HALF
        out[b, :, s:s + HALF, :] = res.results[core]["out"].reshape(64, HALF, W)
    return out


if __name__ == "__main__":
    print("smoke build only")
    _build_program()
    print("build ok")



# revision 2
# speedup vs baseline: 1.3872x; 1.3872x over previous
"""DCNv2 deformable-conv alignment kernel for 8 Trainium2 NeuronCores (v3).

yc-folded d=2 gather, resident source, software pipeline: yc-folded d=2 gather, resident source, software pipeline.

Sharding: core i handles (b = i//2, row-half = i%2) of the B=4, H=128 input.

v3 vs v2 baseline:
- ap_gather costs ~28ns/index on this platform, payload-independent; it is the
  dominant cost (73728 idx/core ~= 2.06ms).  Everything else must hide under it.
- gather d=2 (x-pair only); the y-patch dim moved onto partitions:
  partition p = 16g + 8*yc + c holds channel (g,c)'s image shifted down yc rows.
  Halves payload (~4% faster/idx) and removes the 2x channel duplication
  (w3 has no zero rows; dcn matmuls per tap: 2 instead of 4).
- gather source resident in SBUF (57.4KB/partition, loaded once) - no per-chunk
  band DMA.  Per-chunk slot bias (+4c*NPR) folded into the idx2 activation bias.
- 2-deep software pipeline: emit order [gather(c); offset(c+1); wraps(c+1);
  taps(c)] so conv2/idx/wraps of c+1 execute during gather(c)'s ~120us Q7
  execution and gather(c+1) starts the moment gather(c) completes.
- corner weights: cu4[72,(yc,xc,jj)] products on DVE; per (tap,xc) a 2-matmul
  one-hot rep (disjoint yc partition sets accumulated in one PSUM bank) gives
  cuP[128,jj]; gw = gout*cuP (one DVE mult per (tap,xc)); dcn matmul moving
  operand contiguous.  Output unpermuted (jj->q) for free via the lrelu out AP.
"""
import sys

for _p in ("/opt/trn_rl_repo", "/root/.axon_site/_ro/trn_rl_repo"):
    if _p not in sys.path:
        sys.path.insert(0, _p)

import numpy as np
import ml_dtypes

NF, G, K = 64, 8, 3
KK = K * K
CG = NF // G
B, H, W = 4, 128, 128
N_CORES = 8
HALF = H // 2

GP = 14                 # guard pad rows/cols
WP = W + 2 * GP         # 156
HP = HALF + 2 * GP      # 92
NPR = WP                # pair slots per row (78 A + 78 B)
NE_AB = HP * NPR        # 14352 slots

CH = 512                # positions per chunk (4 rows)
RPC = CH // W           # 4
NCHUNK = HALF * W // CH # 16
NIDX = CH
NIC = KK * NIDX         # 4608 indices per chunk

IDX_ON_ACT = True
FLOOR_TRUNC = False
CAST_ON_ACT = False

_compiled = None


def _build_program():
    import concourse.bacc as bacc
    import concourse.mybir as mybir
    import concourse.tile as tile
    from concourse.tile_rust import add_dep_helper
    from concourse.bass_types import AP

    def mk_ap(base_ap, dims):
        return AP(base_ap.tensor, base_ap.offset, dims)

    dt = mybir.dt
    Alu = mybir.AluOpType
    Act = mybir.ActivationFunctionType
    f32 = dt.float32
    f32r = dt.float32r
    bf16 = dt.bfloat16

    nc = bacc.Bacc("TRN2", target_bir_lowering=False, debug=False,
                   num_devices=N_CORES)

    # ---- DRAM I/O ----
    conv_in_d = nc.dram_tensor("conv_in", [128, 68 * 130], bf16, kind="ExternalInput").ap()
    p2_d = nc.dram_tensor("p2", [128, NE_AB * 2], bf16, kind="ExternalInput").ap()
    w1_d = nc.dram_tensor("w1", [128, KK * 64], bf16, kind="ExternalInput").ap()
    w2_d = nc.dram_tensor("w2", [64, 3 * KK * 72], bf16, kind="ExternalInput").ap()
    w3_d = nc.dram_tensor("w3", [128, KK * 64], bf16, kind="ExternalInput").ap()
    wru_d = nc.dram_tensor("wru", [72, KK * 128], bf16, kind="ExternalInput").ap()
    wrt_d = nc.dram_tensor("wrt", [72, KK * 128], bf16, kind="ExternalInput").ap()
    by_d = nc.dram_tensor("by", [72, 1], f32, kind="ExternalInput").ap()
    bx_d = nc.dram_tensor("bx", [72, 1], f32, kind="ExternalInput").ap()
    bm_d = nc.dram_tensor("bm", [72, 1], f32, kind="ExternalInput").ap()
    b1_d = nc.dram_tensor("b1", [64, 1], f32, kind="ExternalInput").ap()
    b3_d = nc.dram_tensor("b3", [64, 1], f32, kind="ExternalInput").ap()
    e0_d = nc.dram_tensor("e0", [64, 1], f32, kind="ExternalInput").ap()
    e65_d = nc.dram_tensor("e65", [64, 1], f32, kind="ExternalInput").ap()
    rampc_d = nc.dram_tensor("rampc", [128, CH], f32, kind="ExternalInput").ap()
    cbias_d = nc.dram_tensor("cbias", [72, NCHUNK], f32, kind="ExternalInput").ap()
    out_d = nc.dram_tensor("out", [64, HALF * W], f32, kind="ExternalOutput").ap()

    def lrelu_stt(out_ap, in_ap):
        nc.vector.scalar_tensor_tensor(out_ap, in_ap, 0.1, in_ap, Alu.mult, Alu.max)

    # static SBUF for gather-adjacent tensors (partition-strided DMA writes are
    # under-tracked by Tile): fixed addresses + explicit deps
    idxw_s = [nc.alloc_sbuf_tensor(f"idxw{i}", [128, NIC // 16], dt.int16).ap()
              for i in range(2)]
    gout_s = [nc.alloc_sbuf_tensor(f"gout{i}", [128, NIC * 2], bf16).ap()
              for i in range(2)]
    p2_s = nc.alloc_sbuf_tensor("p2s", [128, NE_AB * 2], bf16).ap()

    with tile.TileContext(nc) as tc:
        with tc.tile_pool(name="const", bufs=1) as cpool, \
             tc.tile_pool(name="psum", bufs=2, space="PSUM") as ppool, \
             tc.tile_pool(name="psum_rep", bufs=2, space="PSUM") as ppool_r:

            # ---- persistent loads ----
            w1_sb = cpool.tile([128, KK * 64], bf16)
            nc.sync.dma_start(w1_sb[:], w1_d[:])
            w2_sb = cpool.tile([64, 3 * KK * 72], bf16)
            nc.sync.dma_start(w2_sb[:], w2_d[:])
            w3_sb = cpool.tile([128, KK * 64], bf16)
            nc.sync.dma_start(w3_sb[:], w3_d[:])
            wru_sb = cpool.tile([72, KK * 128], bf16)
            nc.sync.dma_start(wru_sb[:], wru_d[:])
            wrt_sb = cpool.tile([72, KK * 128], bf16)
            nc.sync.dma_start(wrt_sb[:], wrt_d[:])
            by_sb = cpool.tile([72, 1], f32)
            nc.sync.dma_start(by_sb[:], by_d[:])
            bx_sb = cpool.tile([72, 1], f32)
            nc.sync.dma_start(bx_sb[:], bx_d[:])
            bm_sb = cpool.tile([72, 1], f32)
            nc.sync.dma_start(bm_sb[:], bm_d[:])
            b1_sb = cpool.tile([64, 1], f32)
            nc.sync.dma_start(b1_sb[:], b1_d[:])
            b3_sb = cpool.tile([64, 1], f32)
            nc.sync.dma_start(b3_sb[:], b3_d[:])
            e0_sb = cpool.tile([64, 1], f32)
            nc.sync.dma_start(e0_sb[:], e0_d[:])
            e65_sb = cpool.tile([64, 1], f32)
            nc.sync.dma_start(e65_sb[:], e65_d[:])
            rampc_sb = cpool.tile([128, CH], f32)
            nc.sync.dma_start(rampc_sb[:], rampc_d[:])
            cbias_sb = cpool.tile([72, NCHUNK], f32)
            nc.sync.dma_start(cbias_sb[:], cbias_d[:])

            # resident gather source: 4-way split load
            p2_dmas = []
            qd = (NE_AB * 2) // 4
            for si, q in enumerate((nc.sync, nc.scalar, nc.sync, nc.scalar)):
                lo = si * qd
                hi = (si + 1) * qd if si < 3 else NE_AB * 2
                p2_dmas.append(q.dma_start(p2_s[:, lo:hi], p2_d[:, lo:hi]))

            # ---- conv1 -> off_feat rows [-1, HALF+1), 130-wide padded, bf16 ----
            off_sb = cpool.tile([64, 66 * 130], bf16)
            nc.vector.memset(off_sb[:], 0.0)
            off_v = off_sb[:].rearrange("p (r c) -> p r c", c=130)
            with tc.tile_pool(name="cin", bufs=1) as cinpool:
                conv_in_sb = cinpool.tile([128, 68 * 130], bf16)
                nc.sync.dma_start(conv_in_sb[:], conv_in_d[:])
                cin_v = conv_in_sb[:].rearrange("p (r c) -> p r c", c=130)
                j0 = 0
                while j0 < 66:
                    nrow = min(4, 66 - j0)
                    ps1 = ppool.tile([64, nrow, 128], f32, tag="dcn_ps")
                    for kt in range(KK):
                        ky, kx = kt // 3, kt % 3
                        rhs = cin_v[:, j0 + ky: j0 + ky + nrow, kx: kx + 128]
                        nc.tensor.matmul(ps1[:], w1_sb[:, kt * 64:(kt + 1) * 64],
                                         rhs, start=(kt == 0), stop=(kt == KK - 1))
                    scf = cinpool.tile([64, nrow, 128], f32, tag="scf")
                    nc.vector.tensor_scalar(scf[:], ps1[:], b1_sb[:, 0:1], None, Alu.add)
                    lrelu_stt(off_v[:, j0: j0 + nrow, 1:129], scf[:])
                    j0 += nrow
            # zero off rows outside the image (conv2 zero-padding semantics)
            nc.vector.tensor_scalar(off_sb[:, 0:130], off_sb[:, 0:130],
                                    e0_sb[:, 0:1], None, Alu.mult)
            nc.vector.tensor_scalar(off_sb[:, 65 * 130:66 * 130],
                                    off_sb[:, 65 * 130:66 * 130],
                                    e65_sb[:, 0:1], None, Alu.mult)

            # ---- per-chunk software pipeline ----
            gathers, wraps = [], []
            chunk_last_mult = {}
            with tc.tile_pool(name="work", bufs=1) as wpool, \
                 tc.tile_pool(name="work2", bufs=2) as wpool2:

                def cast_int_back(q, tag):
                    ti = wpool.tile([72, CH], dt.int32, tag="fl_i32")
                    tf = wpool.tile([72, CH], f32, tag=tag + "_f")
                    if CAST_ON_ACT:
                        nc.scalar.activation(ti[:], q[:], Act.Identity,
                                             bias=0.0, scale=1.0)
                        nc.scalar.activation(tf[:], ti[:], Act.Identity,
                                             bias=0.0, scale=1.0)
                    else:
                        nc.vector.tensor_copy(ti[:], q[:])
                        nc.vector.tensor_copy(tf[:], ti[:])
                    return tf

                def floor_frac(q, tag):
                    tf = cast_int_back(q, tag)
                    if FLOOR_TRUNC:
                        fl = tf
                    else:
                        gg = wpool.tile([72, CH], f32, tag="fl_gt")
                        nc.vector.tensor_tensor(gg[:], tf[:], q[:], Alu.is_gt)
                        fl = wpool.tile([72, CH], f32, tag=tag)
                        nc.vector.tensor_tensor(fl[:], tf[:], gg[:], Alu.subtract)
                    w = wpool.tile([72, CH], f32, tag=tag + "_w")
                    nc.vector.tensor_tensor(w[:], q[:], fl[:], Alu.subtract)
                    return fl, w

                def emit_offset(c):
                    """conv2 + frac/floor + idx2 + corner-weight products."""
                    outs_f = []
                    for f, (bias_ap, fun, tagn) in enumerate(
                            ((by_sb, Act.Identity, "qy"),
                             (bx_sb, Act.Identity, "qx"),
                             (bm_sb, Act.Sigmoid, "msk"))):
                        psf = ppool.tile([72, RPC, 128], f32, tag="ps2")
                        for kt in range(KK):
                            ky, kx = kt // 3, kt % 3
                            rhs = off_v[:, c * RPC + ky: c * RPC + ky + RPC, kx: kx + 128]
                            nc.tensor.matmul(
                                psf[:],
                                w2_sb[:, (f * KK + kt) * 72:(f * KK + kt + 1) * 72],
                                rhs, start=(kt == 0), stop=(kt == KK - 1))
                        ot = wpool.tile([72, CH], f32, tag=tagn)
                        nc.scalar.activation(ot[:], psf[:].rearrange("p a b -> p (a b)"),
                                             fun, bias=bias_ap[:, 0:1], scale=1.0)
                        outs_f.append(ot)
                    qy, qx, msk = outs_f

                    fyf, wy = floor_frac(qy, "fy")
                    fxf, wx = floor_frac(qx, "fx")

                    # pair index: pidx = fyf*156 + (ramp + fxf*0.5) + 155*parh
                    hx = wpool.tile([72, CH], f32, tag="hx")
                    nc.vector.scalar_tensor_tensor(hx[:], fxf[:], 0.5,
                                                   rampc_sb[:72, :], Alu.mult, Alu.add)
                    pf_ = cast_int_back(hx, "pr")
                    pd = wpool.tile([72, CH], f32, tag="pr_d")
                    nc.vector.tensor_tensor(pd[:], hx[:], pf_[:], Alu.subtract)
                    sq = wpool.tile([72, CH], f32, tag="sq")
                    nc.vector.tensor_tensor(sq[:], pd[:], pd[:], Alu.mult)
                    hxp = wpool.tile([72, CH], f32, tag="hxp")
                    nc.vector.scalar_tensor_tensor(hxp[:], sq[:], float(2 * (NPR - 1)),
                                                   hx[:], Alu.mult, Alu.add)
                    pidx = wpool.tile([72, CH], f32, tag="pidx")
                    nc.vector.scalar_tensor_tensor(pidx[:], fyf[:], float(NPR),
                                                   hxp[:], Alu.mult, Alu.add)

                    # idx2 int16; full-source chunk shift (+4c*NPR) via ACT bias
                    idx2 = wpool2.tile([72, CH], dt.int16, tag="idx2")
                    idx_ins = []
                    d = nc.scalar.activation(idx2[:], pidx[:], Act.Identity,
                                             bias=cbias_sb[:, c:c + 1], scale=1.0)
                    for w in wraps[(c - 2) * KK:(c - 1) * KK] if c >= 2 else []:
                        add_dep_helper(d.ins, w.ins, True, "idx2 WAR wrap")
                    idx_ins.append(d)

                    # corner-weight products cu4[72, (yc,xc)*CH + jj], jj-order
                    t = wpool.tile([72, CH], f32, tag="t")
                    nc.vector.tensor_tensor(t[:], wy[:], msk[:], Alu.mult)
                    uy0 = wpool.tile([72, CH], f32, tag="uy0")
                    nc.vector.tensor_tensor(uy0[:], msk[:], t[:], Alu.subtract)
                    ux0 = wpool.tile([72, CH], f32, tag="ux0")
                    nc.scalar.activation(ux0[:], wx[:], Act.Identity, bias=1.0, scale=-1.0)

                    cu_store = wpool2.tile([72, CH * 4], bf16, tag="cu")
                    for (yc, xc, a_ap, b_ap) in (
                            (0, 0, uy0, ux0), (0, 1, uy0, wx),
                            (1, 0, t, ux0), (1, 1, t, wx)):
                        half = cu_store[:, (yc * 2 + xc) * CH:(yc * 2 + xc + 1) * CH]
                        out_v = half.rearrange("p (xl r xh) -> p xl r xh",
                                               xl=32, r=RPC, xh=4).transpose([0, 2, 3, 1])
                        nc.vector.tensor_tensor(
                            out_v,
                            a_ap[:].rearrange("p (r xh xl) -> p r xh xl",
                                              r=RPC, xh=4),
                            b_ap[:].rearrange("p (r xh xl) -> p r xh xl",
                                              r=RPC, xh=4),
                            Alu.mult)
                    return idx2, cu_store, idx_ins

                def emit_wraps(c, idx2, idx_ins):
                    bi = c % 2
                    grp = []
                    for kt in range(KK):
                        d_b = idxw_s[bi][:, kt * 32:(kt + 1) * 32]
                        s_b = idx2[kt * 8:(kt + 1) * 8, :]
                        q = nc.sync if (len(wraps) % 2 == 0) else nc.scalar
                        wrap = q.dma_start(
                            mk_ap(d_b, [[NIC // 16, 128], [1, 32]]),
                            mk_ap(s_b, [[NIDX, 8], [32, 16], [1, 32]]))
                        for d in idx_ins:
                            add_dep_helper(wrap.ins, d.ins, True, "wrap RAW idx2")
                        if c >= 2:
                            add_dep_helper(wrap.ins, gathers[c - 2].ins, True,
                                           "idxw WAR prev gather")
                        wraps.append(wrap)
                        grp.append(wrap)
                    return grp

                def emit_gather(c, grp_wraps):
                    gth = nc.gpsimd.ap_gather(
                        out_ap=gout_s[c % 2][:], in_ap=p2_s[:],
                        idxs_ap=idxw_s[c % 2][:], channels=128,
                        num_elems=NE_AB, d=2, num_idxs=NIC)
                    for w in grp_wraps:
                        add_dep_helper(gth.ins, w.ins, True, "gather RAW idxw")
                    for dd in p2_dmas:
                        add_dep_helper(gth.ins, dd.ins, True, "gather RAW p2")
                    if c >= 2:
                        add_dep_helper(gth.ins, chunk_last_mult[c - 2].ins, True,
                                       "gout WAR mults 2 back")
                    gathers.append(gth)
                    return gth

                def emit_taps(c, cu_store, gth):
                    gout_v = gout_s[c % 2][:].rearrange(
                        "p (k j x) -> p k j x", k=KK, j=NIDX, x=2)
                    dcn_ps = ppool.tile([64, CH], f32, tag="dcn_ps")
                    for kt in range(KK):
                        for xc in range(2):
                            cup = ppool_r.tile([128, CH], f32, tag="cuP")
                            nc.tensor.matmul(
                                cup[:], wru_sb[:, kt * 128:(kt + 1) * 128],
                                cu_store[:, xc * CH:(xc + 1) * CH],
                                start=True, stop=False)
                            nc.tensor.matmul(
                                cup[:], wrt_sb[:, kt * 128:(kt + 1) * 128],
                                cu_store[:, (2 + xc) * CH:(3 + xc) * CH],
                                start=False, stop=True)
                            gw = wpool2.tile([128, CH], bf16, tag="gw")
                            mult = nc.vector.tensor_tensor(
                                gw[:], gout_v[:, kt, :, xc], cup[:], Alu.mult)
                            add_dep_helper(mult.ins, gth.ins, True, "mult RAW gout")
                            chunk_last_mult[c] = mult
                            nc.tensor.matmul(
                                dcn_ps[:], w3_sb[:, kt * 64:(kt + 1) * 64],
                                gw[:],
                                start=(kt == 0 and xc == 0),
                                stop=(kt == KK - 1 and xc == 1))

                    oc = wpool.tile([64, CH], f32, tag="oc")
                    nc.scalar.activation(oc[:], dcn_ps[:], Act.Identity,
                                         bias=b3_sb[:, 0:1], scale=1.0)
                    # lrelu + jj->q unpermute via APs (out q-order, in jj-order)
                    ob = wpool.tile([64, CH], f32, tag="ob")
                    ob_v = ob[:].rearrange("p (r xh xl) -> p r xh xl",
                                           r=RPC, xh=4, xl=32)
                    oc_v = oc[:].rearrange("p (xl r xh) -> p xl r xh",
                                           xl=32, r=RPC, xh=4).transpose([0, 2, 3, 1])
                    nc.vector.scalar_tensor_tensor(ob_v, oc_v, 0.1, oc_v,
                                                   Alu.mult, Alu.max)
                    nc.sync.dma_start(out_d[:, c * CH:(c + 1) * CH], ob[:])

                # prologue + steady state
                idx2_0, cu_0, ii_0 = emit_offset(0)
                w_0 = emit_wraps(0, idx2_0, ii_0)
                pend = {0: (cu_0, w_0)}
                for c in range(NCHUNK):
                    cu_c, w_c = pend.pop(c)
                    gth = emit_gather(c, w_c)
                    if c + 1 < NCHUNK:
                        idx2_n, cu_n, ii_n = emit_offset(c + 1)
                        w_n = emit_wraps(c + 1, idx2_n, ii_n)
                        pend[c + 1] = (cu_n, w_n)
                    emit_taps(c, cu_c, gth)

    nc.compile()
    return nc


def _prep_inputs(nbr, ref, w_off1, b_off1, w_om, b_om, w_dcn, b_dcn):
    """Build the 8 per-core input dicts."""
    bf = ml_dtypes.bfloat16
    in_maps = []
    # weights shared by all cores
    w1 = np.zeros((128, KK * 64), np.float32)
    for kt in range(KK):
        ky, kx = kt // 3, kt % 3
        w1[:, kt * 64:(kt + 1) * 64] = w_off1[:, :, ky, kx].T
    w2 = np.zeros((64, 3 * KK * 72), np.float32)
    for f in range(3):
        for kt in range(KK):
            ky, kx = kt // 3, kt % 3
            blk = np.zeros((64, 72), np.float32)
            for k in range(KK):
                for g in range(G):
                    blk[:, k * 8 + g] = w_om[f * 72 + g * KK + k, :, ky, kx]
            w2[:, (f * KK + kt) * 72:(f * KK + kt + 1) * 72] = blk
    # w3: partition p = 16g + 8*yc + c (no zero rows)
    w3 = np.zeros((128, KK * 64), np.float32)
    wd = w_dcn.reshape(64, G, CG, 3, 3)
    for kt in range(KK):
        ky, kx = kt // 3, kt % 3
        blk = np.zeros((128, 64), np.float32)
        for g in range(G):
            for yc in range(2):
                for cc in range(CG):
                    blk[16 * g + 8 * yc + cc, :] = wd[:, g, cc, ky, kx]
        w3[:, kt * 64:(kt + 1) * 64] = blk

    # one-hot reps: row kt*8+g -> partitions 16g+c (wru, yc=0) / 16g+8+c (wrt)
    wru = np.zeros((72, KK * 128), np.float32)
    wrt = np.zeros((72, KK * 128), np.float32)
    for kt in range(KK):
        for g in range(G):
            for cc in range(CG):
                wru[kt * 8 + g, kt * 128 + 16 * g + cc] = 1.0
                wrt[kt * 8 + g, kt * 128 + 16 * g + 8 + cc] = 1.0

    dy = np.repeat(np.arange(3) - 1, 3).astype(np.float32)
    dx = np.tile(np.arange(3) - 1, 3).astype(np.float32)
    by = np.zeros((72, 1), np.float32)
    bx = np.zeros((72, 1), np.float32)
    bm = np.zeros((72, 1), np.float32)
    for k in range(KK):
        for g in range(G):
            p = k * 8 + g
            by[p, 0] = b_om[0 * 72 + g * KK + k] + dy[k] + GP
            bx[p, 0] = b_om[1 * 72 + g * KK + k] + dx[k] + GP
            bm[p, 0] = b_om[2 * 72 + g * KK + k]
    b1 = b_off1.reshape(64, 1).astype(np.float32)
    b3 = b_dcn.reshape(64, 1).astype(np.float32)

    q = np.arange(CH, dtype=np.float32)
    rampc = (q // W) * NPR + (q % W) * 0.5
    rampc = np.broadcast_to(rampc, (128, CH)).astype(np.float32).copy()

    for core in range(N_CORES):
        b = core // 2
        s = (core % 2) * HALF
        # conv1 input: concat channels, rows [s-2, s+66), zero pad, 130 cols
        ci = np.zeros((128, 68, 130), np.float32)
        cat = np.concatenate([nbr[b], ref[b]], axis=0)
        r_lo, r_hi = s - 2, s + 66
        src_lo, src_hi = max(r_lo, 0), min(r_hi, H)
        ci[:, src_lo - r_lo: src_hi - r_lo, 1:129] = cat[:, src_lo:src_hi, :]

        # gather source: guard-padded per-channel image rows [s-GP, s+HALF+GP],
        # one extra row so the yc=1 shift stays in range
        padi = np.zeros((64, HP + 1, WP), np.float32)
        g_lo = s - GP
        sg_lo, sg_hi = max(g_lo, 0), min(g_lo + HP + 1, H)
        padi[:, sg_lo - g_lo: sg_hi - g_lo, GP:GP + W] = nbr[b, :, sg_lo:sg_hi, :]
        p2 = np.zeros((128, HP, NPR, 2), np.float32)
        for g in range(G):
            for yc in range(2):
                for cc in range(CG):
                    img = padi[CG * g + cc, yc: yc + HP, :]     # [HP, WP]
                    shift = np.concatenate(
                        [img[:, 1:], np.zeros((HP, 1), np.float32)], axis=1)
                    ab = np.concatenate([img.reshape(HP, NPR // 2, 2),
                                         shift.reshape(HP, NPR // 2, 2)], axis=1)
                    p2[16 * g + 8 * yc + cc] = ab

        e0 = np.full((64, 1), 0.0 if s == 0 else 1.0, np.float32)
        e65 = np.full((64, 1), 0.0 if s + HALF == H else 1.0, np.float32)
        in_maps.append(dict(
            conv_in=ci.reshape(128, -1).astype(bf),
            p2=p2.reshape(128, NE_AB * 2).astype(bf),
            w1=w1.astype(bf), w2=w2.astype(bf), w3=w3.astype(bf),
            wru=wru.astype(bf), wrt=wrt.astype(bf),
            by=by, bx=bx, bm=bm, b1=b1, b3=b3, e0=e0, e65=e65,
            rampc=rampc,
            cbias=np.broadcast_to(
                (np.arange(NCHUNK, dtype=np.float32) * 4 * NPR)[None, :],
                (72, NCHUNK)).copy(),
        ))
    return in_maps


def kernel(**inputs):
    global _compiled
    from concourse.bass_utils import run_bass_kernel_spmd

    if _compiled is None:
        _compiled = _build_program()
    nc = _compiled

    in_maps = _prep_inputs(
        inputs["nbr_fea_l"], inputs["ref_fea_l"], inputs["w_off1"],
        inputs["b_off1"], inputs["w_om"], inputs["b_om"],
        inputs["w_dcn"], inputs["b_dcn"])

    res = run_bass_kernel_spmd(nc, in_maps, core_ids=list(range(N_CORES)))
    out = np.zeros((B, NF, H, W), np.float32)
    for core in range(N_CORES):
        b = core // 2
        s = (core % 2) * HALF
        out[b, :, s:s + HALF, :] = res.results[core]["out"].reshape(64, HALF, W)
    return out


if __name__ == "__main__":
    print("smoke build only")
    _build_program()
    print("build ok")


# revision 3
# speedup vs baseline: 1.3922x; 1.0036x over previous
"""DCNv2 deformable-conv alignment kernel for 8 Trainium2 NeuronCores (v3).

yc-folded d=2 gather, resident source, software pipeline: yc-folded d=2 gather, resident source, software pipeline.

Sharding: core i handles (b = i//2, row-half = i%2) of the B=4, H=128 input.

v3 vs v2 baseline:
- ap_gather costs ~28ns/index on this platform, payload-independent; it is the
  dominant cost (73728 idx/core ~= 2.06ms).  Everything else must hide under it.
- gather d=2 (x-pair only); the y-patch dim moved onto partitions:
  partition p = 16g + 8*yc + c holds channel (g,c)'s image shifted down yc rows.
  Halves payload (~4% faster/idx) and removes the 2x channel duplication
  (w3 has no zero rows; dcn matmuls per tap: 2 instead of 4).
- gather source resident in SBUF (57.4KB/partition, loaded once) - no per-chunk
  band DMA.  Per-chunk slot bias (+4c*NPR) folded into the idx2 activation bias.
- 2-deep software pipeline: emit order [gather(c); offset(c+1); wraps(c+1);
  taps(c)] so conv2/idx/wraps of c+1 execute during gather(c)'s ~120us Q7
  execution and gather(c+1) starts the moment gather(c) completes.
- corner weights: cu4[72,(yc,xc,jj)] products on DVE; per (tap,xc) a 2-matmul
  one-hot rep (disjoint yc partition sets accumulated in one PSUM bank) gives
  cuP[128,jj]; gw = gout*cuP (one DVE mult per (tap,xc)); dcn matmul moving
  operand contiguous.  Output unpermuted (jj->q) for free via the lrelu out AP.
"""
import sys

for _p in ("/opt/trn_rl_repo", "/root/.axon_site/_ro/trn_rl_repo"):
    if _p not in sys.path:
        sys.path.insert(0, _p)

import numpy as np
import ml_dtypes

NF, G, K = 64, 8, 3
KK = K * K
CG = NF // G
B, H, W = 4, 128, 128
N_CORES = 8
HALF = H // 2

GP = 14                 # guard pad rows/cols
WP = W + 2 * GP         # 156
HP = HALF + 2 * GP      # 92
NPR = WP                # pair slots per row (78 A + 78 B)
NE_AB = HP * NPR        # 14352 slots

CH = 512                # positions per chunk (4 rows)
RPC = CH // W           # 4
NCHUNK = HALF * W // CH # 16
NIDX = CH
NIC = KK * NIDX         # 4608 indices per chunk

IDX_ON_ACT = True
FLOOR_TRUNC = False
CAST_ON_ACT = False

_compiled = None


def _build_program():
    import concourse.bacc as bacc
    import concourse.mybir as mybir
    import concourse.tile as tile
    from concourse.tile_rust import add_dep_helper
    from concourse.bass_types import AP

    def mk_ap(base_ap, dims):
        return AP(base_ap.tensor, base_ap.offset, dims)

    dt = mybir.dt
    Alu = mybir.AluOpType
    Act = mybir.ActivationFunctionType
    f32 = dt.float32
    f32r = dt.float32r
    bf16 = dt.bfloat16

    nc = bacc.Bacc("TRN2", target_bir_lowering=False, debug=False,
                   num_devices=N_CORES)

    # ---- DRAM I/O ----
    conv_in_d = nc.dram_tensor("conv_in", [128, 68 * 130], bf16, kind="ExternalInput").ap()
    p2_d = nc.dram_tensor("p2", [128, NE_AB * 2], bf16, kind="ExternalInput").ap()
    w1_d = nc.dram_tensor("w1", [128, KK * 64], bf16, kind="ExternalInput").ap()
    w2_d = nc.dram_tensor("w2", [64, 3 * KK * 72], bf16, kind="ExternalInput").ap()
    w3_d = nc.dram_tensor("w3", [128, KK * 64], bf16, kind="ExternalInput").ap()
    wru_d = nc.dram_tensor("wru", [96, KK * 128], bf16, kind="ExternalInput").ap()
    wrt_d = nc.dram_tensor("wrt", [96, KK * 128], bf16, kind="ExternalInput").ap()
    by_d = nc.dram_tensor("by", [72, 1], f32, kind="ExternalInput").ap()
    bx_d = nc.dram_tensor("bx", [72, 1], f32, kind="ExternalInput").ap()
    bm_d = nc.dram_tensor("bm", [72, 1], f32, kind="ExternalInput").ap()
    b1_d = nc.dram_tensor("b1", [64, 1], f32, kind="ExternalInput").ap()
    b3_d = nc.dram_tensor("b3", [64, 1], f32, kind="ExternalInput").ap()
    e0_d = nc.dram_tensor("e0", [64, 1], f32, kind="ExternalInput").ap()
    e65_d = nc.dram_tensor("e65", [64, 1], f32, kind="ExternalInput").ap()
    rampc_d = nc.dram_tensor("rampc", [128, CH], f32, kind="ExternalInput").ap()
    cbias_d = nc.dram_tensor("cbias", [72, NCHUNK], f32, kind="ExternalInput").ap()
    out_d = nc.dram_tensor("out", [64, HALF * W], f32, kind="ExternalOutput").ap()

    def lrelu_stt(out_ap, in_ap):
        nc.vector.scalar_tensor_tensor(out_ap, in_ap, 0.1, in_ap, Alu.mult, Alu.max)

    # static SBUF for gather-adjacent tensors (partition-strided DMA writes are
    # under-tracked by Tile): fixed addresses + explicit deps
    idxw_s = [nc.alloc_sbuf_tensor(f"idxw{i}", [128, NIC // 16], dt.int16).ap()
              for i in range(2)]
    gout_s = [nc.alloc_sbuf_tensor(f"gout{i}", [128, NIC * 2], bf16).ap()
              for i in range(2)]
    p2_s = nc.alloc_sbuf_tensor("p2s", [128, NE_AB * 2], bf16).ap()

    with tile.TileContext(nc) as tc:
        with tc.tile_pool(name="const", bufs=1) as cpool, \
             tc.tile_pool(name="psum", bufs=2, space="PSUM") as ppool, \
             tc.tile_pool(name="psum_rep", bufs=2, space="PSUM") as ppool_r:

            # ---- persistent loads ----
            w1_sb = cpool.tile([128, KK * 64], bf16)
            nc.sync.dma_start(w1_sb[:], w1_d[:])
            w2_sb = cpool.tile([64, 3 * KK * 72], bf16)
            nc.sync.dma_start(w2_sb[:], w2_d[:])
            w3_sb = cpool.tile([128, KK * 64], bf16)
            nc.sync.dma_start(w3_sb[:], w3_d[:])
            wru_sb = cpool.tile([96, KK * 128], bf16)
            nc.sync.dma_start(wru_sb[:], wru_d[:])
            wrt_sb = cpool.tile([96, KK * 128], bf16)
            nc.sync.dma_start(wrt_sb[:], wrt_d[:])
            by_sb = cpool.tile([72, 1], f32)
            nc.sync.dma_start(by_sb[:], by_d[:])
            bx_sb = cpool.tile([72, 1], f32)
            nc.sync.dma_start(bx_sb[:], bx_d[:])
            bm_sb = cpool.tile([72, 1], f32)
            nc.sync.dma_start(bm_sb[:], bm_d[:])
            b1_sb = cpool.tile([64, 1], f32)
            nc.sync.dma_start(b1_sb[:], b1_d[:])
            b3_sb = cpool.tile([64, 1], f32)
            nc.sync.dma_start(b3_sb[:], b3_d[:])
            e0_sb = cpool.tile([64, 1], f32)
            nc.sync.dma_start(e0_sb[:], e0_d[:])
            e65_sb = cpool.tile([64, 1], f32)
            nc.sync.dma_start(e65_sb[:], e65_d[:])
            rampc_sb = cpool.tile([128, CH], f32)
            nc.sync.dma_start(rampc_sb[:], rampc_d[:])
            cbias_sb = cpool.tile([72, NCHUNK], f32)
            nc.sync.dma_start(cbias_sb[:], cbias_d[:])

            # resident gather source: 4-way split load
            p2_dmas = []
            qd = (NE_AB * 2) // 4
            for si, q in enumerate((nc.sync, nc.scalar, nc.sync, nc.scalar)):
                lo = si * qd
                hi = (si + 1) * qd if si < 3 else NE_AB * 2
                p2_dmas.append(q.dma_start(p2_s[:, lo:hi], p2_d[:, lo:hi]))

            # ---- conv1 -> off_feat rows [-1, HALF+1), 130-wide padded, bf16 ----
            off_sb = cpool.tile([64, 66 * 130], bf16)
            nc.vector.memset(off_sb[:], 0.0)
            off_v = off_sb[:].rearrange("p (r c) -> p r c", c=130)
            with tc.tile_pool(name="cin", bufs=1) as cinpool:
                conv_in_sb = cinpool.tile([128, 68 * 130], bf16)
                nc.sync.dma_start(conv_in_sb[:], conv_in_d[:])
                cin_v = conv_in_sb[:].rearrange("p (r c) -> p r c", c=130)
                j0 = 0
                while j0 < 66:
                    nrow = min(4, 66 - j0)
                    ps1 = ppool.tile([64, nrow, 128], f32, tag="dcn_ps")
                    for kt in range(KK):
                        ky, kx = kt // 3, kt % 3
                        rhs = cin_v[:, j0 + ky: j0 + ky + nrow, kx: kx + 128]
                        nc.tensor.matmul(ps1[:], w1_sb[:, kt * 64:(kt + 1) * 64],
                                         rhs, start=(kt == 0), stop=(kt == KK - 1))
                    scf = cinpool.tile([64, nrow, 128], f32, tag="scf")
                    nc.vector.tensor_scalar(scf[:], ps1[:], b1_sb[:, 0:1], None, Alu.add)
                    lrelu_stt(off_v[:, j0: j0 + nrow, 1:129], scf[:])
                    j0 += nrow
            # zero off rows outside the image (conv2 zero-padding semantics)
            nc.vector.tensor_scalar(off_sb[:, 0:130], off_sb[:, 0:130],
                                    e0_sb[:, 0:1], None, Alu.mult)
            nc.vector.tensor_scalar(off_sb[:, 65 * 130:66 * 130],
                                    off_sb[:, 65 * 130:66 * 130],
                                    e65_sb[:, 0:1], None, Alu.mult)

            # ---- per-chunk software pipeline ----
            gathers, wraps = [], []
            chunk_last_mult = {}
            with tc.tile_pool(name="work", bufs=1) as wpool, \
                 tc.tile_pool(name="work2", bufs=2) as wpool2:

                def cast_int_back(q, tag):
                    ti = wpool.tile([72, CH], dt.int32, tag="fl_i32")
                    tf = wpool.tile([72, CH], f32, tag=tag + "_f")
                    if CAST_ON_ACT:
                        nc.scalar.activation(ti[:], q[:], Act.Identity,
                                             bias=0.0, scale=1.0)
                        nc.scalar.activation(tf[:], ti[:], Act.Identity,
                                             bias=0.0, scale=1.0)
                    else:
                        nc.vector.tensor_copy(ti[:], q[:])
                        nc.vector.tensor_copy(tf[:], ti[:])
                    return tf

                def floor_frac(q, tag):
                    tf = cast_int_back(q, tag)
                    if FLOOR_TRUNC:
                        fl = tf
                    else:
                        gg = wpool.tile([72, CH], f32, tag="fl_gt")
                        nc.vector.tensor_tensor(gg[:], tf[:], q[:], Alu.is_gt)
                        fl = wpool.tile([72, CH], f32, tag=tag)
                        nc.vector.tensor_tensor(fl[:], tf[:], gg[:], Alu.subtract)
                    w = wpool.tile([72, CH], f32, tag=tag + "_w")
                    nc.vector.tensor_tensor(w[:], q[:], fl[:], Alu.subtract)
                    return fl, w

                def emit_offset(c):
                    """conv2 + frac/floor + idx2 + corner-weight products."""
                    outs_f = []
                    for f, (bias_ap, fun, tagn) in enumerate(
                            ((by_sb, Act.Identity, "qy"),
                             (bx_sb, Act.Identity, "qx"),
                             (bm_sb, Act.Sigmoid, "msk"))):
                        psf = ppool.tile([72, RPC, 128], f32, tag="ps2")
                        for kt in range(KK):
                            ky, kx = kt // 3, kt % 3
                            rhs = off_v[:, c * RPC + ky: c * RPC + ky + RPC, kx: kx + 128]
                            nc.tensor.matmul(
                                psf[:],
                                w2_sb[:, (f * KK + kt) * 72:(f * KK + kt + 1) * 72],
                                rhs, start=(kt == 0), stop=(kt == KK - 1))
                        ot = wpool.tile([72, CH], f32, tag=tagn)
                        nc.scalar.activation(ot[:], psf[:].rearrange("p a b -> p (a b)"),
                                             fun, bias=bias_ap[:, 0:1], scale=1.0)
                        outs_f.append(ot)
                    qy, qx, msk = outs_f

                    fyf, wy = floor_frac(qy, "fy")
                    fxf, wx = floor_frac(qx, "fx")

                    # pair index: pidx = fyf*156 + (ramp + fxf*0.5) + 155*parh
                    hx = wpool.tile([72, CH], f32, tag="hx")
                    nc.vector.scalar_tensor_tensor(hx[:], fxf[:], 0.5,
                                                   rampc_sb[:72, :], Alu.mult, Alu.add)
                    pf_ = cast_int_back(hx, "pr")
                    pd = wpool.tile([72, CH], f32, tag="pr_d")
                    nc.vector.tensor_tensor(pd[:], hx[:], pf_[:], Alu.subtract)
                    sq = wpool.tile([72, CH], f32, tag="sq")
                    nc.vector.tensor_tensor(sq[:], pd[:], pd[:], Alu.mult)
                    hxp = wpool.tile([72, CH], f32, tag="hxp")
                    nc.vector.scalar_tensor_tensor(hxp[:], sq[:], float(2 * (NPR - 1)),
                                                   hx[:], Alu.mult, Alu.add)
                    pidx = wpool.tile([72, CH], f32, tag="pidx")
                    nc.vector.scalar_tensor_tensor(pidx[:], fyf[:], float(NPR),
                                                   hxp[:], Alu.mult, Alu.add)

                    # idx2 int16; full-source chunk shift (+4c*NPR) via ACT bias
                    idx2 = wpool2.tile([72, CH], dt.int16, tag="idx2")
                    idx_ins = []
                    d = nc.scalar.activation(idx2[:], pidx[:], Act.Identity,
                                             bias=cbias_sb[:, c:c + 1], scale=1.0)
                    for w in wraps[(c - 2) * KK:(c - 1) * KK] if c >= 2 else []:
                        add_dep_helper(d.ins, w.ins, True, "idx2 WAR wrap")
                    idx_ins.append(d)

                    # corner-weight products cu4[72, (yc,xc)*CH + jj], jj-order
                    t = wpool.tile([72, CH], f32, tag="t")
                    nc.vector.tensor_tensor(t[:], wy[:], msk[:], Alu.mult)
                    uy0 = wpool.tile([72, CH], f32, tag="uy0")
                    nc.vector.tensor_tensor(uy0[:], msk[:], t[:], Alu.subtract)
                    ux0 = wpool.tile([72, CH], f32, tag="ux0")
                    nc.scalar.activation(ux0[:], wx[:], Act.Identity, bias=1.0, scale=-1.0)

                    cu_store = wpool2.tile([96, CH * 4], bf16, tag="cu")
                    for (yc, xc, a_ap, b_ap) in (
                            (0, 0, uy0, ux0), (0, 1, uy0, wx),
                            (1, 0, t, ux0), (1, 1, t, wx)):
                        half = cu_store[:72, (yc * 2 + xc) * CH:(yc * 2 + xc + 1) * CH]
                        out_v = half.rearrange("p (xl r xh) -> p xl r xh",
                                               xl=32, r=RPC, xh=4).transpose([0, 2, 3, 1])
                        nc.vector.tensor_tensor(
                            out_v,
                            a_ap[:].rearrange("p (r xh xl) -> p r xh xl",
                                              r=RPC, xh=4),
                            b_ap[:].rearrange("p (r xh xl) -> p r xh xl",
                                              r=RPC, xh=4),
                            Alu.mult)
                    return idx2, cu_store, idx_ins

                def emit_wraps(c, idx2, idx_ins):
                    bi = c % 2
                    grp = []
                    for kt in range(KK):
                        d_b = idxw_s[bi][:, kt * 32:(kt + 1) * 32]
                        s_b = idx2[kt * 8:(kt + 1) * 8, :]
                        q = nc.sync if (len(wraps) % 2 == 0) else nc.scalar
                        wrap = q.dma_start(
                            mk_ap(d_b, [[NIC // 16, 128], [1, 32]]),
                            mk_ap(s_b, [[NIDX, 8], [32, 16], [1, 32]]))
                        for d in idx_ins:
                            add_dep_helper(wrap.ins, d.ins, True, "wrap RAW idx2")
                        if c >= 2:
                            add_dep_helper(wrap.ins, gathers[c - 2].ins, True,
                                           "idxw WAR prev gather")
                        wraps.append(wrap)
                        grp.append(wrap)
                    return grp

                def emit_gather(c, grp_wraps):
                    gth = nc.gpsimd.ap_gather(
                        out_ap=gout_s[c % 2][:], in_ap=p2_s[:],
                        idxs_ap=idxw_s[c % 2][:], channels=128,
                        num_elems=NE_AB, d=2, num_idxs=NIC)
                    for w in grp_wraps:
                        add_dep_helper(gth.ins, w.ins, True, "gather RAW idxw")
                    for dd in p2_dmas:
                        add_dep_helper(gth.ins, dd.ins, True, "gather RAW p2")
                    if c >= 2:
                        add_dep_helper(gth.ins, chunk_last_mult[c - 2].ins, True,
                                       "gout WAR mults 2 back")
                    gathers.append(gth)
                    return gth

                def emit_taps(c, cu_store, gth):
                    gout_v = gout_s[c % 2][:].rearrange(
                        "p (k j x) -> p k j x", k=KK, j=NIDX, x=2)
                    dcn_ps = ppool.tile([64, CH], f32, tag="dcn_ps")
                    for kt in range(KK):
                        for xc in range(2):
                            base = (kt * 8) // 32 * 32
                            cup = ppool_r.tile([128, CH], f32, tag="cuP")
                            nc.tensor.matmul(
                                cup[:],
                                wru_sb[base:base + 32,
                                       kt * 128:(kt + 1) * 128],
                                cu_store[base:base + 32,
                                         xc * CH:(xc + 1) * CH],
                                start=True, stop=False)
                            nc.tensor.matmul(
                                cup[:],
                                wrt_sb[base:base + 32,
                                       kt * 128:(kt + 1) * 128],
                                cu_store[base:base + 32,
                                         (2 + xc) * CH:(3 + xc) * CH],
                                start=False, stop=True)
                            gw = wpool2.tile([128, CH], bf16, tag="gw")
                            mult = nc.vector.tensor_tensor(
                                gw[:], gout_v[:, kt, :, xc], cup[:], Alu.mult)
                            add_dep_helper(mult.ins, gth.ins, True, "mult RAW gout")
                            chunk_last_mult[c] = mult
                            nc.tensor.matmul(
                                dcn_ps[:], w3_sb[:, kt * 64:(kt + 1) * 64],
                                gw[:],
                                start=(kt == 0 and xc == 0),
                                stop=(kt == KK - 1 and xc == 1))

                    oc = wpool.tile([64, CH], f32, tag="oc")
                    nc.scalar.activation(oc[:], dcn_ps[:], Act.Identity,
                                         bias=b3_sb[:, 0:1], scale=1.0)
                    # lrelu + jj->q unpermute via APs (out q-order, in jj-order)
                    ob = wpool.tile([64, CH], f32, tag="ob")
                    ob_v = ob[:].rearrange("p (r xh xl) -> p r xh xl",
                                           r=RPC, xh=4, xl=32)
                    oc_v = oc[:].rearrange("p (xl r xh) -> p xl r xh",
                                           xl=32, r=RPC, xh=4).transpose([0, 2, 3, 1])
                    nc.vector.scalar_tensor_tensor(ob_v, oc_v, 0.1, oc_v,
                                                   Alu.mult, Alu.max)
                    nc.sync.dma_start(out_d[:, c * CH:(c + 1) * CH], ob[:])

                # prologue + steady state
                idx2_0, cu_0, ii_0 = emit_offset(0)
                w_0 = emit_wraps(0, idx2_0, ii_0)
                pend = {0: (cu_0, w_0)}
                for c in range(NCHUNK):
                    cu_c, w_c = pend.pop(c)
                    gth = emit_gather(c, w_c)
                    if c + 1 < NCHUNK:
                        idx2_n, cu_n, ii_n = emit_offset(c + 1)
                        w_n = emit_wraps(c + 1, idx2_n, ii_n)
                        pend[c + 1] = (cu_n, w_n)
                    emit_taps(c, cu_c, gth)

    nc.compile()
    return nc


def _prep_inputs(nbr, ref, w_off1, b_off1, w_om, b_om, w_dcn, b_dcn):
    """Build the 8 per-core input dicts."""
    bf = ml_dtypes.bfloat16
    in_maps = []
    # weights shared by all cores
    w1 = np.zeros((128, KK * 64), np.float32)
    for kt in range(KK):
        ky, kx = kt // 3, kt % 3
        w1[:, kt * 64:(kt + 1) * 64] = w_off1[:, :, ky, kx].T
    w2 = np.zeros((64, 3 * KK * 72), np.float32)
    for f in range(3):
        for kt in range(KK):
            ky, kx = kt // 3, kt % 3
            blk = np.zeros((64, 72), np.float32)
            for k in range(KK):
                for g in range(G):
                    blk[:, k * 8 + g] = w_om[f * 72 + g * KK + k, :, ky, kx]
            w2[:, (f * KK + kt) * 72:(f * KK + kt + 1) * 72] = blk
    # w3: partition p = 16g + 8*yc + c (no zero rows)
    w3 = np.zeros((128, KK * 64), np.float32)
    wd = w_dcn.reshape(64, G, CG, 3, 3)
    for kt in range(KK):
        ky, kx = kt // 3, kt % 3
        blk = np.zeros((128, 64), np.float32)
        for g in range(G):
            for yc in range(2):
                for cc in range(CG):
                    blk[16 * g + 8 * yc + cc, :] = wd[:, g, cc, ky, kx]
        w3[:, kt * 64:(kt + 1) * 64] = blk

    # one-hot reps padded to 96 rows; per tap both operands are sliced at the
    # same 32-row window base=(kt*8)//32*32 (PE same-base-partition rule)
    wru = np.zeros((96, KK * 128), np.float32)
    wrt = np.zeros((96, KK * 128), np.float32)
    for kt in range(KK):
        for g in range(G):
            for cc in range(CG):
                wru[kt * 8 + g, kt * 128 + 16 * g + cc] = 1.0
                wrt[kt * 8 + g, kt * 128 + 16 * g + 8 + cc] = 1.0

    dy = np.repeat(np.arange(3) - 1, 3).astype(np.float32)
    dx = np.tile(np.arange(3) - 1, 3).astype(np.float32)
    by = np.zeros((72, 1), np.float32)
    bx = np.zeros((72, 1), np.float32)
    bm = np.zeros((72, 1), np.float32)
    for k in range(KK):
        for g in range(G):
            p = k * 8 + g
            by[p, 0] = b_om[0 * 72 + g * KK + k] + dy[k] + GP
            bx[p, 0] = b_om[1 * 72 + g * KK + k] + dx[k] + GP
            bm[p, 0] = b_om[2 * 72 + g * KK + k]
    b1 = b_off1.reshape(64, 1).astype(np.float32)
    b3 = b_dcn.reshape(64, 1).astype(np.float32)

    q = np.arange(CH, dtype=np.float32)
    rampc = (q // W) * NPR + (q % W) * 0.5
    rampc = np.broadcast_to(rampc, (128, CH)).astype(np.float32).copy()

    for core in range(N_CORES):
        b = core // 2
        s = (core % 2) * HALF
        # conv1 input: concat channels, rows [s-2, s+66), zero pad, 130 cols
        ci = np.zeros((128, 68, 130), np.float32)
        cat = np.concatenate([nbr[b], ref[b]], axis=0)
        r_lo, r_hi = s - 2, s + 66
        src_lo, src_hi = max(r_lo, 0), min(r_hi, H)
        ci[:, src_lo - r_lo: src_hi - r_lo, 1:129] = cat[:, src_lo:src_hi, :]

        # gather source: guard-padded per-channel image rows [s-GP, s+HALF+GP],
        # one extra row so the yc=1 shift stays in range
        padi = np.zeros((64, HP + 1, WP), np.float32)
        g_lo = s - GP
        sg_lo, sg_hi = max(g_lo, 0), min(g_lo + HP + 1, H)
        padi[:, sg_lo - g_lo: sg_hi - g_lo, GP:GP + W] = nbr[b, :, sg_lo:sg_hi, :]
        p2 = np.zeros((128, HP, NPR, 2), np.float32)
        for g in range(G):
            for yc in range(2):
                for cc in range(CG):
                    img = padi[CG * g + cc, yc: yc + HP, :]     # [HP, WP]
                    shift = np.concatenate(
                        [img[:, 1:], np.zeros((HP, 1), np.float32)], axis=1)
                    ab = np.concatenate([img.reshape(HP, NPR // 2, 2),
                                         shift.reshape(HP, NPR // 2, 2)], axis=1)
                    p2[16 * g + 8 * yc + cc] = ab

        e0 = np.full((64, 1), 0.0 if s == 0 else 1.0, np.float32)
        e65 = np.full((64, 1), 0.0 if s + HALF == H else 1.0, np.float32)
        in_maps.append(dict(
            conv_in=ci.reshape(128, -1).astype(bf),
            p2=p2.reshape(128, NE_AB * 2).astype(bf),
            w1=w1.astype(bf), w2=w2.astype(bf), w3=w3.astype(bf),
            wru=wru.astype(bf), wrt=wrt.astype(bf),
            by=by, bx=bx, bm=bm, b1=b1, b3=b3, e0=e0, e65=e65,
            rampc=rampc,
            cbias=np.broadcast_to(
                (np.arange(NCHUNK, dtype=np.float32) * 4 * NPR)[None, :],
                (72, NCHUNK)).copy(),
        ))
    return in_maps


def kernel(**inputs):
    global _compiled
    from concourse.bass_utils import run_bass_kernel_spmd

    if _compiled is None:
        _compiled = _build_program()
    nc = _compiled

    in_maps = _prep_inputs(
        inputs["nbr_fea_l"], inputs["ref_fea_l"], inputs["w_off1"],
        inputs["b_off1"], inputs["w_om"], inputs["b_om"],
        inputs["w_dcn"], inputs["b_dcn"])

    res = run_bass_kernel_spmd(nc, in_maps, core_ids=list(range(N_CORES)))
    out = np.zeros((B, NF, H, W), np.float32)
    for core in range(N_CORES):
        b = core // 2
        s = (core % 2) * HALF
        out[b, :, s:s + HALF, :] = res.results[core]["out"].reshape(64, HALF, W)
    return out


if __name__ == "__main__":
    print("smoke build only")
    _build_program()
    print("build ok")
